# revision 25
# baseline (speedup 1.0000x reference)
"""Trainium2 Bass kernel for nn_GAT_77704548319854.

Math: every per-edge quantity in the reference depends only on the edge's
source node (rows = ent_embed[src], then row-wise ops / linear layers), so the
attention-weighted segment mean collapses exactly:
    h_ent[n] = (cnt[n] * e[n] * c[n]) / (cnt[n] * e[n]) = c[n]   if cnt[n] > 0
    h_ent[n] = 0                                                  if cnt[n] == 0
with c = clip_rownorm(ent_embed) @ W_a.T + b_a. So
    out[n] = relu(c[n]) * 1{n appears in triplets[:,0]}.

Device plan (8 cores, SPMD, node-range sharding):
  - the host buckets edge source ids by node range (core c owns nodes
    [c*12544, (c+1)*12544)) and ships each core its bucket as ready-made
    int16 local row indices. Pure index arithmetic — the membership
    computation itself stays on device, and no cross-core collective is
    needed: a core's bucket covers exactly its node slice.
  - each core builds its 12544-node membership mask with one dma_scatter_add
    pass. The bucket is split into two node-row halves (nodes [0,6272) /
    [6272,12544) == out partitions [0,64) / [64,128)); each half owns a
    private maskbuf region (6272 node rows + 128 dump rows for padding), so
    Tile sees the halves as independent: the lower half's mask extraction,
    apply and store all overlap the upper half's scatter window. Within a
    half, chunks rotate over maskbuf columns 0..3 so consecutive chunks
    carry no WAW dependency (duplicate-index add races are safe for a mask
    since the value stays > 0); the mask is min(col0+..+col3, 1).
  - dense part (rownorm clip, bulk PE transpose, PE-only matmul pairs with
    the bias as an accumulating 1-row matmul, relu) runs concurrently with
    the scatter pass in bf16 on DVE/PE/Act; the mask lands as per-half
    broadcast multiplies (relu(c) >= 0, mask in {0,1}).
  - emb/out ship in the SBUF-resident [128, tiles*64] layout (bf16) so the
    big DMAs are contiguous and halved.
"""
import sys

sys.path.insert(0, "/opt/trn_rl_repo")

import numpy as np

import concourse.bacc as bacc
import concourse.bass as bass
import concourse.mybir as mybir
import concourse.tile as tile
from concourse.bass_utils import run_bass_kernel_spmd
from concourse.masks import make_identity

F = 64          # in_dim == out_dim == 64
N_CORES = 8
NPC = 12544     # nodes per core (128 * 98), 8 * 12544 = 100352 >= 100000
HALF = NPC // 2
HREG = HALF + 128                            # maskbuf rows per half (+dump)


class Cfg:
    """Each core's edge bucket is split into two node-row halves; each half
    is padded to k full chunks plus one small tail chunk (the tail keeps
    both the capacity slack and the trailing DMA short)."""

    def __init__(self, k=14, chunk=7168, tail=2048):
        # chunks above 7168 idxs crash the Q7 scatter ucode (8192 fails on
        # HW; 7168 verified bit-identical to 6272)
        self.chunk = chunk
        self.half_chunks = [chunk] * k + ([tail] if tail else [])
        self.hc = sum(self.half_chunks)      # capacity per half
        self.ecp = 2 * self.hc               # padded idxs per core
        self.tiles = NPC // 128              # 98 [128,64] node tiles per core
        self.np_total = N_CORES * NPC


# edges per (core, half): mean 1.6e6 * 6272/1e5 = 100352, sigma ~ 300;
# capacity 14*7168 + 2048 = 102400 is ~6.8 sigma of headroom (and kernel()
# rebuilds with a larger config if an input ever exceeds it).
FULL = Cfg()

f32 = mybir.dt.float32
bf16 = mybir.dt.bfloat16
i16 = mybir.dt.int16


def build(cfg: Cfg, n_cores=N_CORES):
    tiles = cfg.tiles
    ids_cols = cfg.ecp // 16

    nc = bacc.Bacc("TRN2", target_bir_lowering=False, debug=False,
                   num_devices=n_cores)
    ids_d = nc.dram_tensor("ids", [16, ids_cols], i16, kind="ExternalInput")
    emb_d = nc.dram_tensor("emb", [128, tiles * F], bf16,
                           kind="ExternalInput")
    wa_d = nc.dram_tensor("wa", [F, F], bf16, kind="ExternalInput")
    ba_d = nc.dram_tensor("ba", [1, F], bf16, kind="ExternalInput")
    out_d = nc.dram_tensor("out", [128, tiles * F], bf16,
                           kind="ExternalOutput")

    with tile.TileContext(nc) as tc:
        with tc.tile_pool(name="sb", bufs=1) as sb, \
             tc.tile_pool(name="sbt", bufs=2) as sbt, \
             tc.tile_pool(name="ps", bufs=2, space="PSUM") as ps, \
             tc.tile_pool(name="dram", bufs=1, space="DRAM") as dram:

            # ======== phase A: membership mask via one scatter-add pass ====
            maskbuf = dram.tile([2 * HREG, F], f32, name="mb")
            half_ap = [maskbuf[h * HREG:(h + 1) * HREG, :] for h in range(2)]

            # edge row indices: partitions 0..31 carry the two replicas the
            # Q7 tx/rx pair reads; partitions 32+ are never read by queue 0's
            # Q7 pair, so they stay uninitialized. Loaded first so chunk 0's
            # desc-gen can start while the zero-fill DMAs are in flight.
            # split per half so chunk 0 gates only on the lower half's ids
            # and zero-fill; zero cols 0..3 of the node rows (the only bytes
            # scatter writes and extraction reads; dump rows are never read)
            idx16 = sb.tile([128, ids_cols], i16)
            zt = sb.tile([128, (HALF // 128) * 4], f32)
            nc.vector.memset(zt[:], 0.0)
            hcols = ids_cols // 2
            for h in range(2):
                cs = slice(h * hcols, (h + 1) * hcols)
                for g in range(2):
                    nc.sync.dma_start(out=idx16[16 * g:16 * (g + 1), cs],
                                      in_=ids_d[:, cs])
                nc.sync.dma_start(
                    out=half_ap[h][0:HALF, 0:4]
                        .rearrange("(p t) f -> p t f", p=128),
                    in_=zt[:].rearrange("p (t f) -> p t f", f=4))

            ones = sb.tile([128, cfg.chunk // 128], f32)
            nc.vector.memset(ones[:], 1.0)

            # all lower-half chunks first, then all upper-half chunks, so the
            # lower half's tail (phase C) overlaps the upper half's scatter
            coff = 0
            for h in range(2):
                for i, sz in enumerate(cfg.half_chunks):
                    col = i % 4
                    nc.gpsimd.dma_scatter_add(
                        half_ap[h][:, col:col + 1],
                        ones[:, :sz // 128][:, :, None],
                        idx16[:, coff:coff + sz // 16],
                        sz, sz, 1, elem_step=F)
                    coff += sz // 16

            # ======== phase B: dense per-node compute (overlaps phase A) ===
            emb_sb = sb.tile([128, tiles * F], bf16)
            nc.sync.dma_start(out=emb_sb[:], in_=emb_d[:])
            emb3 = emb_sb[:].rearrange("p (t f) -> p t f", f=F)
            out_sb = sb.tile([128, tiles * F], bf16)
            sq = sb.tile([128, tiles * F], f32)
            nc.vector.tensor_mul(out=sq[:], in0=emb_sb[:], in1=emb_sb[:])
            ssq = sb.tile([128, tiles], f32)
            nc.vector.tensor_reduce(out=ssq[:],
                                    in_=sq[:].rearrange("p (t f) -> p t f",
                                                        f=F),
                                    axis=mybir.AxisListType.X,
                                    op=mybir.AluOpType.add)
            nrm = sb.tile([128, tiles], f32)
            nc.scalar.sqrt(out=nrm[:], in_=ssq[:])
            nc.vector.tensor_scalar_add(out=nrm[:], in0=nrm[:], scalar1=1e-7)
            rec = sb.tile([128, tiles], f32)
            nc.vector.reciprocal(out=rec[:], in_=nrm[:])
            recb = sb.tile([128, tiles], bf16)
            nc.vector.tensor_scalar_min(out=recb[:], in0=rec[:], scalar1=1.0)
            # h = emb * scale (broadcast scale along features)
            nc.vector.tensor_tensor(
                out=emb3, in0=emb3,
                in1=recb[:][:, :, None].to_broadcast([128, tiles, F]),
                op=mybir.AluOpType.mult)

            ident = sb.tile([128, 128], bf16)
            make_identity(nc, ident[:])
            wat_sb = sb.tile([F, F], bf16)   # host ships W_a already
            nc.sync.dma_start(out=wat_sb[:], in_=wa_d[:])  # transposed
            ba_sb = sb.tile([1, F], bf16)
            nc.sync.dma_start(out=ba_sb[:], in_=ba_d[:])
            ones1 = sb.tile([1, 128], bf16)
            nc.vector.memset(ones1[:], 1.0)

            # bulk-transpose h into SBUF first (PE runs ahead, DVE copies
            # trail), then PE-only matmul pairs; this keeps every engine's
            # in-order queue free of cross-engine round-trips per tile
            htall = sb.tile([F, tiles * 128], bf16)
            for t in range(tiles):
                h_t = emb_sb[:, t * F:(t + 1) * F]
                ht_ps = ps.tile([F, 128], bf16, tag="ht", bufs=4)
                nc.tensor.transpose(out=ht_ps[:], in_=h_t, identity=ident[:])
                nc.vector.tensor_copy(out=htall[:, t * 128:(t + 1) * 128],
                                      in_=ht_ps[:])

            relu = mybir.ActivationFunctionType.Relu
            for t in range(tiles):
                c_ps = ps.tile([128, F], f32, tag="cps", bufs=3)
                nc.tensor.matmul(c_ps[:], htall[:, t * 128:(t + 1) * 128],
                                 wat_sb[:], start=True, stop=False)
                nc.tensor.matmul(c_ps[:], ones1[:], ba_sb[:],
                                 start=False, stop=True)
                nc.scalar.activation(out=out_sb[:, t * F:(t + 1) * F],
                                     in_=c_ps[:], func=relu)

            # ======== phase C: extract mask, apply, store (per half) ======
            # node rows [h*6272, (h+1)*6272) are exactly out partitions
            # [h*64, (h+1)*64). Issued at the lowest priority so the Tile
            # scheduler keeps it behind the dense work in the in-order DVE
            # stream; the lower half's instance only waits on the lower
            # half's scatters, so it runs during the upper half's window.
            with tc.high_priority(offset=-(1 << 20)):
                # full-height tiles sliced per half, so every elementwise op
                # sees both SBUF inputs at the same base partition (a walrus
                # verifier requirement)
                mext = sbt.tile([128, tiles * 4], f32, tag="mext")
                s01 = sbt.tile([128, tiles], f32, tag="s01")
                s23 = sbt.tile([128, tiles], f32, tag="s23")
                msum = sbt.tile([128, tiles], f32, tag="msum")
                mask = sbt.tile([128, tiles], bf16, tag="mask")
                for h in range(2):
                    pl = slice(64 * h, 64 * (h + 1))
                    # structural ordering guard: pre-write mext from this
                    # half's tail of out_sb (written by the LAST activations
                    # of the dense pipeline), so no scheduler model can place
                    # the extraction chain — and with it this half's DVE mask
                    # ops — before the dense work in the in-order DVE stream
                    nc.vector.tensor_copy(out=mext[pl, :],
                                          in_=out_sb[pl, -tiles * 4:])
                    nc.sync.dma_start(
                        out=mext[pl, :].rearrange("p (t f) -> p t f", f=4),
                        in_=half_ap[h][0:HALF, 0:4]
                            .rearrange("(p t) f -> p t f", p=64))
                    mext3 = mext[pl, :].rearrange("p (t f) -> p t f", f=4)
                    nc.vector.tensor_tensor(out=s01[pl, :],
                                            in0=mext3[:, :, 0],
                                            in1=mext3[:, :, 1],
                                            op=mybir.AluOpType.add)
                    nc.vector.tensor_tensor(out=s23[pl, :],
                                            in0=mext3[:, :, 2],
                                            in1=mext3[:, :, 3],
                                            op=mybir.AluOpType.add)
                    nc.vector.tensor_tensor(out=msum[pl, :], in0=s01[pl, :],
                                            in1=s23[pl, :],
                                            op=mybir.AluOpType.add)
                    nc.vector.tensor_scalar_min(out=mask[pl, :],
                                                in0=msum[pl, :],
                                                scalar1=1.0)
                    # mask-multiply and store in four segments so each store
                    # DMA overlaps the next multiply (matters for the upper
                    # half, whose tail is the exposed end of the kernel)
                    q = tiles // 4 + 1
                    for lo in range(0, tiles, q):
                        hi = min(lo + q, tiles)
                        seg = slice(lo * F, hi * F)
                        nc.vector.tensor_tensor(
                            out=out_sb[pl, seg]
                                .rearrange("p (t f) -> p t f", f=F),
                            in0=out_sb[pl, seg]
                                .rearrange("p (t f) -> p t f", f=F),
                            in1=mask[pl, lo:hi][:, :, None]
                                .to_broadcast([64, hi - lo, F]),
                            op=mybir.AluOpType.mult)
                        nc.sync.dma_start(out=out_d[pl, seg],
                                          in_=out_sb[pl, seg])

    nc.compile()
    return nc


_cache = {}


def _get_nc(cfg: Cfg = FULL):
    key = (cfg.ecp, cfg.chunk)
    if key not in _cache:
        _cache[key] = build(cfg)
    return _cache[key]


def _pad_half(s, hc, rng_start):
    """Pad a half's local row indices to capacity with spread dump rows."""
    sp = np.empty(hc, np.int16)
    sp[:s.shape[0]] = s
    npad = hc - s.shape[0]
    if npad:
        sp[s.shape[0]:] = (HALF
                           + ((rng_start + np.arange(npad)) % 128)
                           ).astype(np.int16)
    return sp


def _in_maps(cfg: Cfg, triplets, ent_embed, W_a, b_a):
    src = np.ascontiguousarray(np.asarray(triplets)[:, 0]).astype(np.int64)
    bucket = src // NPC
    local = (src - bucket * NPC).astype(np.int16)
    sub = bucket * 2 + (local >= HALF)       # (core, half) sub-bucket
    counts = np.bincount(sub, minlength=2 * N_CORES)
    order = np.argsort(sub, kind="stable")
    ls = local[order] % HALF                 # row index within the half
    offs = np.zeros(2 * N_CORES + 1, np.int64)
    np.cumsum(counts, out=offs[1:])

    n = ent_embed.shape[0]
    emb_pad = np.zeros((cfg.np_total, F), np.float32)
    emb_pad[:n] = np.asarray(ent_embed, np.float32)
    bft = mybir.dt.np(mybir.dt.bfloat16)
    wa = np.ascontiguousarray(np.asarray(W_a, np.float32).T).astype(bft)
    ba = np.asarray(b_a, np.float32).reshape(1, F).astype(bft)

    # per-chunk blocks are contiguous runs of the padded stream, laid out so
    # the kernel's column slice [:, coff:coff+sz//16] covers exactly block c
    maps = []
    for c in range(N_CORES):
        halves = []
        for h in range(2):
            s = ls[offs[2 * c + h]:offs[2 * c + h + 1]]
            assert s.shape[0] <= cfg.hc, "bucket overflow; rebuild larger"
            halves.append(_pad_half(s, cfg.hc, rng_start=c * 31 + h * 7))
        flat = np.concatenate(halves)
        blocks = []
        off = 0
        for sz in cfg.half_chunks * 2:
            blocks.append(flat[off:off + sz].reshape(16, sz // 16))
            off += sz
        ids = np.concatenate(blocks, axis=1)
        emb_c = emb_pad[c * NPC:(c + 1) * NPC]
        maps.append({
            "ids": np.ascontiguousarray(ids),
            "emb": np.ascontiguousarray(
                emb_c.reshape(128, cfg.tiles * F)).astype(bft),
            "wa": wa,
            "ba": ba,
        })
    return maps


def kernel(triplets, ent_embed, W_a, b_a, W_a2, b_a2):
    # W_a2 / b_a2 cancel algebraically (see module docstring)
    src = np.asarray(triplets)[:, 0]
    src64 = np.asarray(src, np.int64)
    sub = (src64 // NPC) * 2 + ((src64 % NPC) >= HALF)
    counts = np.bincount(sub, minlength=2 * N_CORES)
    cfg = FULL
    if counts.max() > cfg.hc:
        cfg = Cfg(k=int(np.ceil(counts.max() / FULL.chunk)), tail=0)
    nc = _get_nc(cfg)
    maps = _in_maps(cfg, triplets, ent_embed, W_a, b_a)
    res = run_bass_kernel_spmd(nc, maps, core_ids=list(range(N_CORES)))
    out = np.concatenate(
        [np.asarray(r["out"]).astype(np.float32).reshape(NPC, F)
         for r in res.results], axis=0)
    return np.ascontiguousarray(out[:ent_embed.shape[0]])
